# revision 35
# baseline (speedup 1.0000x reference)
"""Multi-head attention forward on 8 TRN2 NeuronCores.

Sharding: tensor-parallel over heads (4 groups of 4 heads) x data-parallel
over batch (2). Core c: batch c//4, heads [4*(c%4), 4*(c%4)+4).
Each 4-core batch group ReduceScatters the projection partials (bf16, 8
chunks of 256 rows, overlapped with compute) so every core ends with
disjoint [512, 1024] slices of the final output; the host reassembles.

Compute layout is feature-major (transposed) throughout:
  qkvT = W_shard^T @ x^T          [768, T]   (PE, bf16 in / f32 psum)
  S^T  = kT^T qT per k-tile pair  [128, 1024] psum (two 512-col halves)
  P^T  = exp(S^T / 64)            (ScalarE; no max-subtraction needed:
                                   scores have sigma ~0.125)
  O_aug^T = V_aug^T @ P^T accum   [65, 512]  (V_aug has a ones column so
                                   row 64 accumulates the softmax denom)
  epilogue: approx-reciprocal of the [1, 512] denom row (DVE), broadcast
  across 64 partitions with a tiny fp16 PE matmul into o_ps, DVE multiply
  -> O_all^T rows
  y = O_all^T^T @ W_proj          [128, 512] psum tiles

Schedule: ALL 128 attention (head, q-chunk, k-pair) iterations form one
flat software pipeline: iteration i emits S(i), exp(i), then the O-pair
of iteration i-2 — the 2-iteration lag keeps the exp chain saturated
(exp never waits on the S psum drain) and the PE continuously busy so it
holds its fast DVFS p-state. All non-attention PE work (QKV blocks, V
tiles, proj halves) is statically interleaved as filler, ordered by
input-DMA arrival. Queue placement keeps slow waits off compute-critical
queues: y_bounce DMAs on Sync (nothing else mid-kernel), RS triggers and
out-DMAs on GpSimd (all ReduceScatter-chain-bound), a 6-deep y_sb ring
so DVE never waits on a y-DMA.
"""
import os
import sys
import types

import numpy as np

if "/opt/trn_rl_repo" not in sys.path:
    sys.path.insert(0, "/opt/trn_rl_repo")

import concourse.bass as bass
import concourse.bacc as bacc
import concourse.tile as tile
import concourse.mybir as mybir
from concourse import masks
from concourse.bass_utils import run_bass_kernel_spmd

B, T, D = 2, 2048, 1024
H, HD = 16, 64
N_CORES = 8
GROUPS = [[0, 1, 2, 3], [4, 5, 6, 7]]
HPC = 4                 # heads per core
DSH = HPC * HD          # 256 per-core head features
QKV_COLS = 3 * DSH      # 768
TQC = 512               # q-chunk
N_TQ = T // TQC         # 4
N_KT = T // 128         # 16 k-tiles
N_J = N_KT // 2         # 8 k-tile pairs per head-chunk
RS_CHUNKS = [(256 * i, 256) for i in range(8)]

f32 = mybir.dt.float32
bf16 = mybir.dt.bfloat16

LAST_EXEC_NS = None
_CACHE = {}

# Static filler schedule: (tq, h, j) -> units emitted after iteration j of
# attention head-chunk (tq, h).
#   ('v', i)       V k-tile pair kt=2i,2i+1 (4096 stream cols)
#   ('qkv', m, tc) qkvT 128-row block m, t-chunk tc (4096 cols)
#   ('pa', r)      first half of projection rows [r*128, r*128+128)
#   ('pb', r)      second half + y DMA + reduce-scatter poll
# Ordering constraints: m2 tc c before S j=2c of t0 (and x tc c's DMA
# arrival); V pair i before the O-pair of flat iteration i+2; m3 before
# (t0, h2); m0/m1 tc c before the chunks that read q from t-chunk c;
# proj rows r after the epilogues covering them.
FILL = {
    (0, 0, 0): [('v', 1), ('qkv', 3, 0)],
    (0, 0, 1): [('qkv', 2, 1), ('qkv', 1, 0)],
    (0, 0, 2): [('v', 2)],
    (0, 0, 3): [('qkv', 2, 2), ('v', 3)],
    (0, 0, 4): [('v', 4)],
    (0, 0, 5): [('qkv', 2, 3), ('v', 5)],
    (0, 0, 6): [('v', 6)],
    (0, 0, 7): [('v', 7)],
    (0, 1, 0): [('qkv', 3, 1)],
    (0, 1, 2): [('qkv', 3, 2)],
    (0, 1, 4): [('qkv', 3, 3)],
    (0, 1, 6): [('qkv', 0, 1)],
    (0, 2, 0): [('qkv', 1, 1)],
    (0, 3, 0): [('qkv', 0, 2)],
    (1, 0, 2): [('pa', 0)], (1, 0, 6): [('pb', 0)],
    (1, 1, 2): [('pa', 1)], (1, 1, 6): [('pb', 1)],
    (1, 2, 2): [('pa', 2)], (1, 2, 6): [('pb', 2)],
    (1, 3, 2): [('pa', 3)], (1, 3, 4): [('qkv', 1, 2)],
    (1, 3, 6): [('pb', 3)],
    (2, 0, 2): [('pa', 4)], (2, 0, 6): [('pb', 4)],
    (2, 1, 2): [('pa', 5)], (2, 1, 4): [('qkv', 0, 3)],
    (2, 1, 6): [('pb', 5)],
    (2, 2, 2): [('pa', 6)], (2, 2, 6): [('pb', 6)],
    (2, 3, 2): [('pa', 7)], (2, 3, 4): [('qkv', 1, 3)],
    (2, 3, 6): [('pb', 7)],
    (3, 0, 2): [('pa', 8)], (3, 0, 6): [('pb', 8)],
    (3, 1, 2): [('pa', 9)], (3, 1, 6): [('pb', 9)],
    (3, 2, 2): [('pa', 10)], (3, 2, 6): [('pb', 10)],
    (3, 3, 2): [('pa', 11)], (3, 3, 6): [('pb', 11)],
}


def _build():
    nc = bacc.Bacc("TRN2", target_bir_lowering=False, debug=False,
                   num_devices=N_CORES)
    xT_ext = nc.dram_tensor("xT", [D, T], bf16, kind="ExternalInput")
    wqkv_ext = nc.dram_tensor("w_qkv", [D, QKV_COLS], bf16,
                              kind="ExternalInput")
    wproj_ext = nc.dram_tensor("w_proj", [DSH, D], bf16, kind="ExternalInput")
    out_ext = nc.dram_tensor("out", [T // 4, D], bf16, kind="ExternalOutput")
    Exp = mybir.ActivationFunctionType.Exp

    with tile.TileContext(nc) as tc:
        with (
            tc.tile_pool(name="persist", bufs=1) as persist,
            tc.tile_pool(name="dram", bufs=1, space="DRAM") as drampool,
            tc.tile_pool(name="ps_s0", bufs=1, space="PSUM") as pss0,
            tc.tile_pool(name="ps_s1", bufs=1, space="PSUM") as pss1,
            tc.tile_pool(name="ps_o", bufs=2, space="PSUM") as pso,
            tc.tile_pool(name="ps_f", bufs=2, space="PSUM") as psf,
            tc.tile_pool(name="attn", bufs=4) as apool,
            tc.tile_pool(name="ppool", bufs=6) as ppool,
            tc.tile_pool(name="attn2", bufs=2) as apool2,
            tc.tile_pool(name="ypool", bufs=6) as ypool,
        ):
            fp16 = mybir.dt.float16
            qkvT = persist.tile([128, 4, T], bf16)   # q,k rows m*128+p of [512,T]
            wproj = persist.tile([128, 2, D], bf16)  # rows of [256, 1024]
            oallT = persist.tile([128, 2, T], bf16)  # rows of O_all^T [256, T]
            vaug = persist.tile([128, HPC, N_KT, 80], bf16)
            # per-t-chunk x tiles so DMA completion gates at chunk granularity
            xTc = [persist.tile([128, 8, TQC], bf16, name=f"xTc{i}")
                   for i in range(N_TQ)]
            wq_k = persist.tile([128, 8, 2 * 128], bf16)  # k cols (m=2,3)
            wq_q = persist.tile([128, 8, 2 * 128], bf16)  # q cols (m=0,1)
            wq_v = persist.tile([128, 8, DSH], bf16)      # v cols
            # 64-partition broadcast matmul operands: lhsT has a single 1.0
            # row so out[i, j] = rhs[0, j]; rhs rows 1..63 stay zeroed.
            ones64 = persist.tile([HD, HD], fp16)
            rinv64 = persist.tile([HD, TQC], fp16)

            y_bounce = drampool.tile([T, D], bf16, tag="ybounce")

            # ---- input DMAs, priority-ordered for earliest first matmul ----
            wq_src = wqkv_ext.ap().rearrange("(k p) m -> p k m", p=128)
            xT_src = xT_ext.ap().rearrange("(k p) t -> p k t", p=128)
            nc.sync.dma_start(wq_k[:], wq_src[:, :, 256:512])
            nc.sync.dma_start(xTc[0][:], xT_src[:, :, 0:TQC])
            nc.sync.dma_start(wq_q[:], wq_src[:, :, 0:256])
            nc.sync.dma_start(wq_v[:], wq_src[:, :, 512:768])
            for tch in range(1, N_TQ):
                nc.sync.dma_start(xTc[tch][:],
                                  xT_src[:, :, tch * TQC:(tch + 1) * TQC])
            nc.sync.dma_start(
                wproj[:], wproj_ext.ap().rearrange("(c p) d -> p c d", p=128))
            nc.gpsimd.memset(vaug[:], 1.0)
            nc.gpsimd.memset(ones64[:], 0.0)
            nc.gpsimd.memset(ones64[0:1, :], 1.0)
            nc.gpsimd.memset(rinv64[:], 0.0)

            # ---- PE work units ----
            def qkv_m_tc(m, tch):
                """One 128-row block m of qkvT for one 512-col t-chunk."""
                w = wq_k if m >= 2 else wq_q
                mb = (m - 2 if m >= 2 else m) * 128
                t0 = tch * TQC
                ps = psf.tile([128, TQC], f32, tag="f")
                for k in range(8):
                    nc.tensor.matmul(
                        ps[:],
                        w[:, k, mb:mb + 128],
                        xTc[tch][:, k, :],
                        start=(k == 0), stop=(k == 7),
                    )
                nc.vector.tensor_copy(qkvT[:, m, t0:t0 + TQC], ps[:])

            def v_pair(i):
                """V (in [T, hd] orientation) for k-tiles 2i, 2i+1."""
                ps = psf.tile([128, TQC], f32, tag="f")
                for half in range(2):
                    kt = 2 * i + half
                    tch, tb = kt // 4, (kt % 4) * 128
                    base = half * DSH
                    for k in range(8):
                        nc.tensor.matmul(
                            ps[:, base:base + DSH],
                            xTc[tch][:, k, tb:tb + 128],
                            wq_v[:, k, :],
                            start=(k == 0), stop=(k == 7),
                        )
                for half in range(2):
                    kt = 2 * i + half
                    nc.vector.tensor_copy(
                        vaug[:, :, kt, 0:HD],
                        ps[:, half * DSH:(half + 1) * DSH].rearrange(
                            "p (h d) -> p h d", d=HD))

            rs_next = [0, 0]

            def rs_poll(done_rows):
                while rs_next[0] < len(RS_CHUNKS):
                    base, rows = RS_CHUNKS[rs_next[0]]
                    if base + rows > done_rows:
                        break
                    share = rows // 4
                    rs_out = drampool.tile(
                        [share, D], bf16, tag=f"rs{rs_next[0]}")
                    nc.gpsimd.collective_compute(
                        "ReduceScatter", mybir.AluOpType.add,
                        replica_groups=GROUPS,
                        ins=[y_bounce[base:base + rows, :]],
                        outs=[rs_out[:]],
                    )
                    # out-DMA waits on the RS; issue from GpSimd (with the
                    # triggers, all RS-chain-bound) so Sync stays free for
                    # y_bounce DMAs.
                    nc.gpsimd.dma_start(
                        out_ext.ap()[rs_next[1]:rs_next[1] + share, :],
                        rs_out[:])
                    rs_next[0] += 1
                    rs_next[1] += share

            ysb_pend = {}

            def proj_half(r, nn2):
                """Half of the projection for output rows [r*128, +128):
                output features [nn2*512, +512)."""
                r0 = r * 128
                if nn2 == 0:
                    ysb_pend[r] = ypool.tile([128, D], bf16,
                                             tag="ysb", name="y_sb")
                y_sb = ysb_pend[r]
                y_ps = psf.tile([128, TQC], f32, tag="f")
                for kc in range(2):
                    nc.tensor.matmul(
                        y_ps[:],
                        oallT[:, kc, r0:r0 + 128],
                        wproj[:, kc, nn2 * 512:(nn2 + 1) * 512],
                        start=(kc == 0), stop=(kc == 1),
                    )
                nc.vector.tensor_copy(
                    y_sb[:, nn2 * 512:(nn2 + 1) * 512], y_ps[:])
                if nn2 == 1:
                    del ysb_pend[r]
                    nc.sync.dma_start(y_bounce[r0:r0 + 128, :], y_sb[:])
                    rs_poll(r0 + 128)

            def run_unit(u):
                if u[0] == 'v':
                    v_pair(u[1])
                elif u[0] == 'qkv':
                    qkv_m_tc(u[1], u[2])
                elif u[0] == 'pa':
                    proj_half(u[1], 0)
                else:
                    proj_half(u[1], 1)

            pend_epi = [None]

            def flush_epi():
                if pend_epi[0] is not None:
                    f, pend_epi[0] = pend_epi[0], None
                    f()

            def close_hc(tq, h, o_ps):
                """Emit the psum->sbuf copy of the finished O and queue the
                deferred normalization (reciprocal now on DVE; broadcast
                matmul + multiply flushed a pipeline iteration later)."""
                q0 = tq * TQC
                om, op = h // 2, (h % 2) * 64
                o_sb = apool.tile([HD + 1, TQC], bf16, tag="osb")
                nc.vector.tensor_copy(o_sb[:], o_ps[:])
                rrow = apool2.tile([1, TQC], f32, tag="rrow")
                nc.vector.tensor_copy(rrow[:], o_ps[HD:HD + 1, :])
                rinv = apool2.tile([1, TQC], f32, tag="rinv")
                nc.vector.reciprocal_approx_fast(out=rinv[:], in_=rrow[:])
                nc.vector.tensor_copy(rinv64[0:1, :], rinv[:])

                def fin():
                    nc.tensor.matmul(o_ps[0:HD, :], ones64[:],
                                     rinv64[:], start=True, stop=True)
                    nc.vector.tensor_tensor(
                        out=oallT[op:op + HD, om, q0:q0 + TQC],
                        in0=o_sb[0:HD, :], in1=o_ps[0:HD, :],
                        op=mybir.AluOpType.mult)
                pend_epi[0] = fin

            # ---- prefix: minimum PE work before attention can start ----
            qkv_m_tc(2, 0)   # k rows for heads 0,1; t-chunk 0
            qkv_m_tc(0, 0)   # q rows for heads 0,1; chunk 0
            v_pair(0)        # V k-tiles 0,1

            # ---- flat attention pipeline over all (chunk, head, k-pair) ----
            flat = [(tq, h, j)
                    for tq in range(N_TQ) for h in range(HPC)
                    for j in range(N_J)]
            o_tiles = {}
            pend_O = []

            def emit_O(tq, h, j):
                o_ps, p2 = o_tiles[(tq, h)], pend_p2.pop((tq, h, j))
                for half in range(2):
                    kt = 2 * j + half
                    nc.tensor.matmul(
                        o_ps[:], vaug[:, h, kt, 0:HD + 1],
                        p2[:, half * TQC:(half + 1) * TQC],
                        start=(kt == 0), stop=(kt == N_KT - 1),
                    )
                if j == N_J - 1:
                    close_hc(tq, h, o_tiles.pop((tq, h)))

            pend_p2 = {}
            it_count = 0
            for tq, h, j in flat:
                flush_epi()
                q0 = tq * TQC
                qm, qp = h // 2, (h % 2) * 64
                km = 2 + h // 2
                if j == 0:
                    o_tiles[(tq, h)] = pso.tile([HD + 1, TQC], f32,
                                                tag="o", name="o_ps")
                pss = pss0 if it_count % 2 == 0 else pss1
                it_count += 1
                s2 = pss.tile([128, 2 * TQC], f32, tag="s", name="s2")
                for half in range(2):
                    kt = 2 * j + half
                    nc.tensor.matmul(
                        s2[:, half * TQC:(half + 1) * TQC],
                        qkvT[qp:qp + HD, km, kt * 128:(kt + 1) * 128],
                        qkvT[qp:qp + HD, qm, q0:q0 + TQC],
                        start=True, stop=True,
                    )
                p2 = ppool.tile([128, 2 * TQC], bf16, tag="p")
                nc.scalar.activation(p2[:], s2[:], Exp, scale=1.0 / HD)
                pend_p2[(tq, h, j)] = p2
                pend_O.append((tq, h, j))
                if len(pend_O) > 2:
                    emit_O(*pend_O.pop(0))
                for u in FILL.get((tq, h, j), []):
                    run_unit(u)

            # ---- drain the pipeline, last chunk's proj, final RS ----
            while pend_O:
                emit_O(*pend_O.pop(0))
                flush_epi()
            flush_epi()
            for r in (12, 13, 14, 15):
                proj_half(r, 0)
                proj_half(r, 1)

    nc.compile()
    return nc


def _install_profile_hook():
    """Provide antenv.axon_hooks (absent in this image) so bass_utils'
    axon trace path can reach the NTFF profiler in libaxon_pjrt.so."""
    try:
        import antenv
        if "antenv.axon_hooks" not in sys.modules:
            mod = types.ModuleType("antenv.axon_hooks")
            mod._hook = None
            mod.set_axon_ntff_profile_hook = lambda h: setattr(mod, "_hook", h)
            mod.get_axon_ntff_profile_hook = lambda: mod._hook
            sys.modules["antenv.axon_hooks"] = mod
            antenv.axon_hooks = mod
        from trn_agent_boot.trn_boot import _ntff_profile_via_ctypes
        hook = _ntff_profile_via_ctypes("/opt/axon/libaxon_pjrt.so")
        sys.modules["antenv.axon_hooks"].set_axon_ntff_profile_hook(hook)
        return True
    except Exception:
        return False


def kernel(x, W_qkv, W_proj):
    global LAST_EXEC_NS
    x = np.asarray(x, dtype=np.float32)
    W_qkv = np.asarray(W_qkv, dtype=np.float32)
    W_proj = np.asarray(W_proj, dtype=np.float32)

    if "nc" not in _CACHE:
        _CACHE["nc"] = _build()
    nc = _CACHE["nc"]

    npbf16 = mybir.dt.np(bf16)
    xT = [np.ascontiguousarray(x[b].T).astype(npbf16) for b in range(B)]
    in_maps = []
    for c in range(N_CORES):
        b, g = c // 4, c % 4
        wq = W_qkv[:, g * DSH:(g + 1) * DSH]
        wk = W_qkv[:, D + g * DSH:D + (g + 1) * DSH]
        wv = W_qkv[:, 2 * D + g * DSH:2 * D + (g + 1) * DSH]
        in_maps.append({
            "xT": xT[b],
            "w_qkv": np.concatenate([wq, wk, wv], axis=1).astype(npbf16),
            "w_proj": np.ascontiguousarray(
                W_proj[g * DSH:(g + 1) * DSH, :]).astype(npbf16),
        })

    profile = bool(os.environ.get("BASS_KERNEL_PROFILE"))
    trace_dir = os.environ.get("BASS_KERNEL_TRACE_DIR") or None
    if profile:
        profile = _install_profile_hook()
    res = run_bass_kernel_spmd(
        nc, in_maps, core_ids=list(range(N_CORES)),
        trace=profile, tmpdir=trace_dir)
    LAST_EXEC_NS = res.exec_time_ns

    y = np.empty((B, T, D), dtype=np.float32)
    for c in range(N_CORES):
        b, r = c // 4, c % 4
        oc = res.results[c]["out"].astype(np.float32)
        o = 0
        for base, rows in RS_CHUNKS:
            share = rows // 4
            y[b, base + r * share:base + (r + 1) * share, :] = oc[o:o + share]
            o += share
    return y


# revision 38
# speedup vs baseline: 1.0155x; 1.0155x over previous
"""Multi-head attention forward on 8 TRN2 NeuronCores.

Sharding: tensor-parallel over heads (4 groups of 4 heads) x data-parallel
over batch (2). Core c: batch c//4, heads [4*(c%4), 4*(c%4)+4).
Each 4-core batch group ReduceScatters the projection partials (bf16, 8
chunks of 256 rows, overlapped with compute) so every core ends with
disjoint [512, 1024] slices of the final output; the host reassembles.

Compute layout is feature-major (transposed) throughout:
  qkvT = W_shard^T @ x^T          [768, T]   (PE, bf16 in / f32 psum)
  S^T  = kT^T qT per k-tile pair  [128, 1024] psum (two 512-col halves)
  P^T  = exp(S^T / 64)            (ScalarE; no max-subtraction needed:
                                   scores have sigma ~0.125)
  O_aug^T = V_aug^T @ P^T accum   [65, 512]  (V_aug has a ones column so
                                   row 64 accumulates the softmax denom)
  epilogue: approx-reciprocal of the [1, 512] denom row (DVE), broadcast
  across 64 partitions with a tiny fp16 PE matmul into o_ps, DVE multiply
  -> O_all^T rows
  y = O_all^T^T @ W_proj          [128, 512] psum tiles

Schedule: ALL 128 attention (head, q-chunk, k-pair) iterations form one
flat software pipeline: iteration i emits S(i), exp(i), then the O-pair
of iteration i-2 — the 2-iteration lag keeps the exp chain saturated
(exp never waits on the S psum drain) and the PE continuously busy so it
holds its fast DVFS p-state. All non-attention PE work (QKV blocks, V
tiles, proj halves) is statically interleaved as filler, ordered by
input-DMA arrival. Queue placement keeps slow waits off compute-critical
queues: y_bounce DMAs on Sync (nothing else mid-kernel), RS triggers and
out-DMAs on GpSimd (all ReduceScatter-chain-bound), a 6-deep y_sb ring
so DVE never waits on a y-DMA.
"""
import os
import sys
import types

import numpy as np

if "/opt/trn_rl_repo" not in sys.path:
    sys.path.insert(0, "/opt/trn_rl_repo")

import concourse.bass as bass
import concourse.bacc as bacc
import concourse.tile as tile
import concourse.mybir as mybir
from concourse import masks
from concourse.bass_utils import run_bass_kernel_spmd

B, T, D = 2, 2048, 1024
H, HD = 16, 64
N_CORES = 8
GROUPS = [[0, 1, 2, 3], [4, 5, 6, 7]]
HPC = 4                 # heads per core
DSH = HPC * HD          # 256 per-core head features
QKV_COLS = 3 * DSH      # 768
TQC = 512               # q-chunk
N_TQ = T // TQC         # 4
N_KT = T // 128         # 16 k-tiles
N_J = N_KT // 2         # 8 k-tile pairs per head-chunk
RS_CHUNKS = [(256 * i, 256) for i in range(8)]

f32 = mybir.dt.float32
bf16 = mybir.dt.bfloat16

LAST_EXEC_NS = None
_CACHE = {}

# Static filler schedule: (tq, h, j) -> units emitted after iteration j of
# attention head-chunk (tq, h).
#   ('v', i)       V k-tile pair kt=2i,2i+1 (4096 stream cols)
#   ('qkv', m, tc) qkvT 128-row block m, t-chunk tc (4096 cols)
#   ('pa', r)      first half of projection rows [r*128, r*128+128)
#   ('pb', r)      second half + y DMA + reduce-scatter poll
# Ordering constraints: m2 tc c before S j=2c of t0 (and x tc c's DMA
# arrival); V pair i before the O-pair of flat iteration i+2; m3 before
# (t0, h2); m0/m1 tc c before the chunks that read q from t-chunk c;
# proj rows r after the epilogues covering them.
FILL = {
    (0, 0, 0): [('v', 1), ('qkv', 3, 0)],
    (0, 0, 1): [('qkv', 2, 1), ('qkv', 1, 0)],
    (0, 0, 2): [('v', 2)],
    (0, 0, 3): [('qkv', 2, 2), ('v', 3)],
    (0, 0, 4): [('v', 4)],
    (0, 0, 5): [('qkv', 2, 3), ('v', 5)],
    (0, 0, 6): [('v', 6)],
    (0, 0, 7): [('v', 7)],
    (0, 1, 0): [('qkv', 3, 1)],
    (0, 1, 2): [('qkv', 3, 2)],
    (0, 1, 4): [('qkv', 3, 3)],
    (0, 1, 6): [('qkv', 0, 1)],
    (0, 2, 0): [('qkv', 1, 1)],
    (0, 3, 0): [('qkv', 0, 2)],
    (1, 0, 4): [('pa', 0)], (1, 0, 6): [('pb', 0)],
    (1, 1, 4): [('pa', 1)], (1, 1, 6): [('pb', 1)],
    (1, 2, 4): [('pa', 2)], (1, 2, 6): [('pb', 2)],
    (1, 3, 2): [('qkv', 1, 2)], (1, 3, 4): [('pa', 3)],
    (1, 3, 6): [('pb', 3)],
    (2, 0, 4): [('pa', 4)], (2, 0, 6): [('pb', 4)],
    (2, 1, 2): [('qkv', 0, 3)], (2, 1, 4): [('pa', 5)],
    (2, 1, 6): [('pb', 5)],
    (2, 2, 4): [('pa', 6)], (2, 2, 6): [('pb', 6)],
    (2, 3, 2): [('qkv', 1, 3)], (2, 3, 4): [('pa', 7)],
    (2, 3, 6): [('pb', 7)],
    (3, 0, 4): [('pa', 8)], (3, 0, 6): [('pb', 8)],
    (3, 1, 4): [('pa', 9)], (3, 1, 6): [('pb', 9)],
    (3, 2, 4): [('pa', 10)], (3, 2, 6): [('pb', 10)],
    (3, 3, 4): [('pa', 11)], (3, 3, 6): [('pb', 11)],
}


def _build():
    nc = bacc.Bacc("TRN2", target_bir_lowering=False, debug=False,
                   num_devices=N_CORES)
    xT_ext = nc.dram_tensor("xT", [D, T], bf16, kind="ExternalInput")
    wqkv_ext = nc.dram_tensor("w_qkv", [D, QKV_COLS], bf16,
                              kind="ExternalInput")
    wproj_ext = nc.dram_tensor("w_proj", [DSH, D], bf16, kind="ExternalInput")
    out_ext = nc.dram_tensor("out", [T // 4, D], bf16, kind="ExternalOutput")
    Exp = mybir.ActivationFunctionType.Exp

    with tile.TileContext(nc) as tc:
        with (
            tc.tile_pool(name="persist", bufs=1) as persist,
            tc.tile_pool(name="dram", bufs=1, space="DRAM") as drampool,
            tc.tile_pool(name="ps_s", bufs=2, space="PSUM") as pss,
            tc.tile_pool(name="ps_o", bufs=2, space="PSUM") as pso,
            tc.tile_pool(name="ps_f", bufs=2, space="PSUM") as psf,
            tc.tile_pool(name="attn", bufs=4) as apool,
            tc.tile_pool(name="ppool", bufs=6) as ppool,
            tc.tile_pool(name="attn2", bufs=2) as apool2,
            tc.tile_pool(name="ypool", bufs=6) as ypool,
        ):
            fp16 = mybir.dt.float16
            qkvT = persist.tile([128, 4, T], bf16)   # q,k rows m*128+p of [512,T]
            wproj = persist.tile([128, 2, D], bf16)  # rows of [256, 1024]
            oallT = persist.tile([128, 2, T], bf16)  # rows of O_all^T [256, T]
            vaug = persist.tile([128, HPC, N_KT, 80], bf16)
            # per-t-chunk x tiles so DMA completion gates at chunk granularity
            xTc = [persist.tile([128, 8, TQC], bf16, name=f"xTc{i}")
                   for i in range(N_TQ)]
            wq_k = persist.tile([128, 8, 2 * 128], bf16)  # k cols (m=2,3)
            wq_q = persist.tile([128, 8, 2 * 128], bf16)  # q cols (m=0,1)
            wq_v = persist.tile([128, 8, DSH], bf16)      # v cols
            # 64-partition broadcast matmul operands: lhsT has a single 1.0
            # row so out[i, j] = rhs[0, j]; rhs rows 1..63 stay zeroed.
            ones64 = persist.tile([HD, HD], fp16)
            rinv64 = persist.tile([HD, TQC], fp16)

            y_bounce = drampool.tile([T, D], bf16, tag="ybounce")

            # ---- input DMAs, priority-ordered for earliest first matmul ----
            wq_src = wqkv_ext.ap().rearrange("(k p) m -> p k m", p=128)
            xT_src = xT_ext.ap().rearrange("(k p) t -> p k t", p=128)
            nc.sync.dma_start(wq_k[:], wq_src[:, :, 256:512])
            nc.sync.dma_start(xTc[0][:], xT_src[:, :, 0:TQC])
            nc.sync.dma_start(wq_q[:], wq_src[:, :, 0:256])
            nc.sync.dma_start(wq_v[:], wq_src[:, :, 512:768])
            for tch in range(1, N_TQ):
                nc.sync.dma_start(xTc[tch][:],
                                  xT_src[:, :, tch * TQC:(tch + 1) * TQC])
            nc.sync.dma_start(
                wproj[:], wproj_ext.ap().rearrange("(c p) d -> p c d", p=128))
            nc.gpsimd.memset(vaug[:], 1.0)
            nc.gpsimd.memset(ones64[:], 0.0)
            nc.gpsimd.memset(ones64[0:1, :], 1.0)
            nc.gpsimd.memset(rinv64[:], 0.0)

            # ---- PE work units ----
            def qkv_m_tc(m, tch):
                """One 128-row block m of qkvT for one 512-col t-chunk."""
                w = wq_k if m >= 2 else wq_q
                mb = (m - 2 if m >= 2 else m) * 128
                t0 = tch * TQC
                ps = psf.tile([128, TQC], f32, tag="f")
                for k in range(8):
                    nc.tensor.matmul(
                        ps[:],
                        w[:, k, mb:mb + 128],
                        xTc[tch][:, k, :],
                        start=(k == 0), stop=(k == 7),
                    )
                nc.vector.tensor_copy(qkvT[:, m, t0:t0 + TQC], ps[:])

            def v_pair(i):
                """V (in [T, hd] orientation) for k-tiles 2i, 2i+1."""
                ps = psf.tile([128, TQC], f32, tag="f")
                for half in range(2):
                    kt = 2 * i + half
                    tch, tb = kt // 4, (kt % 4) * 128
                    base = half * DSH
                    for k in range(8):
                        nc.tensor.matmul(
                            ps[:, base:base + DSH],
                            xTc[tch][:, k, tb:tb + 128],
                            wq_v[:, k, :],
                            start=(k == 0), stop=(k == 7),
                        )
                for half in range(2):
                    kt = 2 * i + half
                    nc.vector.tensor_copy(
                        vaug[:, :, kt, 0:HD],
                        ps[:, half * DSH:(half + 1) * DSH].rearrange(
                            "p (h d) -> p h d", d=HD))

            rs_next = [0, 0]

            def rs_poll(done_rows):
                while rs_next[0] < len(RS_CHUNKS):
                    base, rows = RS_CHUNKS[rs_next[0]]
                    if base + rows > done_rows:
                        break
                    share = rows // 4
                    rs_out = drampool.tile(
                        [share, D], bf16, tag=f"rs{rs_next[0]}")
                    nc.gpsimd.collective_compute(
                        "ReduceScatter", mybir.AluOpType.add,
                        replica_groups=GROUPS,
                        ins=[y_bounce[base:base + rows, :]],
                        outs=[rs_out[:]],
                    )
                    # out-DMA waits on the RS; issue from GpSimd (with the
                    # triggers, all RS-chain-bound) so Sync stays free for
                    # y_bounce DMAs.
                    nc.gpsimd.dma_start(
                        out_ext.ap()[rs_next[1]:rs_next[1] + share, :],
                        rs_out[:])
                    rs_next[0] += 1
                    rs_next[1] += share

            ysb_pend = {}

            def proj_half(r, nn2):
                """Half of the projection for output rows [r*128, +128):
                output features [nn2*512, +512)."""
                r0 = r * 128
                if nn2 == 0:
                    ysb_pend[r] = ypool.tile([128, D], bf16,
                                             tag="ysb", name="y_sb")
                y_sb = ysb_pend[r]
                y_ps = psf.tile([128, TQC], f32, tag="f")
                for kc in range(2):
                    nc.tensor.matmul(
                        y_ps[:],
                        oallT[:, kc, r0:r0 + 128],
                        wproj[:, kc, nn2 * 512:(nn2 + 1) * 512],
                        start=(kc == 0), stop=(kc == 1),
                    )
                nc.vector.tensor_copy(
                    y_sb[:, nn2 * 512:(nn2 + 1) * 512], y_ps[:])
                if nn2 == 1:
                    del ysb_pend[r]
                    nc.sync.dma_start(y_bounce[r0:r0 + 128, :], y_sb[:])
                    rs_poll(r0 + 128)

            def run_unit(u):
                if u[0] == 'v':
                    v_pair(u[1])
                elif u[0] == 'qkv':
                    qkv_m_tc(u[1], u[2])
                elif u[0] == 'pa':
                    proj_half(u[1], 0)
                else:
                    proj_half(u[1], 1)

            pend_epi = [None]

            def flush_epi():
                if pend_epi[0] is not None:
                    f, pend_epi[0] = pend_epi[0], None
                    f()

            def close_hc(tq, h, o_ps):
                """Emit the psum->sbuf copy of the finished O and queue the
                deferred normalization (reciprocal now on DVE; broadcast
                matmul + multiply flushed a pipeline iteration later)."""
                q0 = tq * TQC
                om, op = h // 2, (h % 2) * 64
                o_sb = apool.tile([HD + 1, TQC], bf16, tag="osb")
                nc.vector.tensor_copy(o_sb[:], o_ps[:])
                rrow = apool2.tile([1, TQC], f32, tag="rrow")
                nc.vector.tensor_copy(rrow[:], o_ps[HD:HD + 1, :])
                rinv = apool2.tile([1, TQC], f32, tag="rinv")
                nc.vector.reciprocal_approx_fast(out=rinv[:], in_=rrow[:])
                nc.vector.tensor_copy(rinv64[0:1, :], rinv[:])

                def fin():
                    nc.tensor.matmul(o_ps[0:HD, :], ones64[:],
                                     rinv64[:], start=True, stop=True)
                    nc.vector.tensor_tensor(
                        out=oallT[op:op + HD, om, q0:q0 + TQC],
                        in0=o_sb[0:HD, :], in1=o_ps[0:HD, :],
                        op=mybir.AluOpType.mult)
                pend_epi[0] = fin

            # ---- prefix: minimum PE work before attention can start ----
            qkv_m_tc(2, 0)   # k rows for heads 0,1; t-chunk 0
            qkv_m_tc(0, 0)   # q rows for heads 0,1; chunk 0
            v_pair(0)        # V k-tiles 0,1

            # ---- flat attention pipeline over all (chunk, head, k-pair) ----
            flat = [(tq, h, j)
                    for tq in range(N_TQ) for h in range(HPC)
                    for j in range(N_J)]
            o_tiles = {}
            pend_O = []

            def emit_O(tq, h, j):
                o_ps, p2 = o_tiles[(tq, h)], pend_p2.pop((tq, h, j))
                for half in range(2):
                    kt = 2 * j + half
                    nc.tensor.matmul(
                        o_ps[:], vaug[:, h, kt, 0:HD + 1],
                        p2[:, half * TQC:(half + 1) * TQC],
                        start=(kt == 0), stop=(kt == N_KT - 1),
                    )
                if j == N_J - 1:
                    close_hc(tq, h, o_tiles.pop((tq, h)))

            pend_p2 = {}

            def s_exp(tq, h, j):
                """S-pair matmuls for k-tile pair j + the exp consuming it."""
                q0 = tq * TQC
                qm, qp = h // 2, (h % 2) * 64
                km = 2 + h // 2
                s2 = pss.tile([128, 2 * TQC], f32, tag="s", name="s2")
                for half in range(2):
                    kt = 2 * j + half
                    nc.tensor.matmul(
                        s2[:, half * TQC:(half + 1) * TQC],
                        qkvT[qp:qp + HD, km, kt * 128:(kt + 1) * 128],
                        qkvT[qp:qp + HD, qm, q0:q0 + TQC],
                        start=True, stop=True,
                    )
                p2 = ppool.tile([128, 2 * TQC], bf16, tag="p")
                nc.scalar.activation(p2[:], s2[:], Exp, scale=1.0 / HD)
                pend_p2[(tq, h, j)] = p2
                pend_O.append((tq, h, j))

            # Two k-pairs per step: grouping the four S matmuls (and the four
            # O matmuls of the lagged pairs) minimizes PSUM-tile context
            # switches on the PE, which cost ~100ns each.
            for tq, h, j in flat:
                if j % 2 == 1:
                    continue
                flush_epi()
                if j == 0:
                    o_tiles[(tq, h)] = pso.tile([HD + 1, TQC], f32,
                                                tag="o", name="o_ps")
                s_exp(tq, h, j)
                s_exp(tq, h, j + 1)
                while len(pend_O) > 3:
                    emit_O(*pend_O.pop(0))
                for jj in (j, j + 1):
                    for u in FILL.get((tq, h, jj), []):
                        run_unit(u)

            # ---- drain the pipeline, last chunk's proj, final RS ----
            while pend_O:
                emit_O(*pend_O.pop(0))
                flush_epi()
            flush_epi()
            for r in (12, 13, 14, 15):
                proj_half(r, 0)
                proj_half(r, 1)

    nc.compile()
    return nc


def _install_profile_hook():
    """Provide antenv.axon_hooks (absent in this image) so bass_utils'
    axon trace path can reach the NTFF profiler in libaxon_pjrt.so."""
    try:
        import antenv
        if "antenv.axon_hooks" not in sys.modules:
            mod = types.ModuleType("antenv.axon_hooks")
            mod._hook = None
            mod.set_axon_ntff_profile_hook = lambda h: setattr(mod, "_hook", h)
            mod.get_axon_ntff_profile_hook = lambda: mod._hook
            sys.modules["antenv.axon_hooks"] = mod
            antenv.axon_hooks = mod
        from trn_agent_boot.trn_boot import _ntff_profile_via_ctypes
        hook = _ntff_profile_via_ctypes("/opt/axon/libaxon_pjrt.so")
        sys.modules["antenv.axon_hooks"].set_axon_ntff_profile_hook(hook)
        return True
    except Exception:
        return False


def kernel(x, W_qkv, W_proj):
    global LAST_EXEC_NS
    x = np.asarray(x, dtype=np.float32)
    W_qkv = np.asarray(W_qkv, dtype=np.float32)
    W_proj = np.asarray(W_proj, dtype=np.float32)

    if "nc" not in _CACHE:
        _CACHE["nc"] = _build()
    nc = _CACHE["nc"]

    npbf16 = mybir.dt.np(bf16)
    xT = [np.ascontiguousarray(x[b].T).astype(npbf16) for b in range(B)]
    in_maps = []
    for c in range(N_CORES):
        b, g = c // 4, c % 4
        wq = W_qkv[:, g * DSH:(g + 1) * DSH]
        wk = W_qkv[:, D + g * DSH:D + (g + 1) * DSH]
        wv = W_qkv[:, 2 * D + g * DSH:2 * D + (g + 1) * DSH]
        in_maps.append({
            "xT": xT[b],
            "w_qkv": np.concatenate([wq, wk, wv], axis=1).astype(npbf16),
            "w_proj": np.ascontiguousarray(
                W_proj[g * DSH:(g + 1) * DSH, :]).astype(npbf16),
        })

    profile = bool(os.environ.get("BASS_KERNEL_PROFILE"))
    trace_dir = os.environ.get("BASS_KERNEL_TRACE_DIR") or None
    if profile:
        profile = _install_profile_hook()
    res = run_bass_kernel_spmd(
        nc, in_maps, core_ids=list(range(N_CORES)),
        trace=profile, tmpdir=trace_dir)
    LAST_EXEC_NS = res.exec_time_ns

    y = np.empty((B, T, D), dtype=np.float32)
    for c in range(N_CORES):
        b, r = c // 4, c % 4
        oc = res.results[c]["out"].astype(np.float32)
        o = 0
        for base, rows in RS_CHUNKS:
            share = rows // 4
            y[b, base + r * share:base + (r + 1) * share, :] = oc[o:o + share]
            o += share
    return y
